# revision 1
# baseline (speedup 1.0000x reference)
"""TRN2 Bass kernel for nn_CausalSelfAttention_4054449128214.

The reference returns out_s + stop_gradient(out_full - out_s), whose forward
value is exactly out_full — plain dense causal self-attention. So the kernel
computes: qkv = x@W_attn+b_attn, per-head causal softmax attention, y@W_proj+b_proj.

Sharding (8 cores, no collectives):
  Megatron head-parallel. Cores 0-3 own head pairs (0,1)..(6,7); cores 4-7 own
  heads 8..11 (run twice for SPMD shape-uniformity, second copy's W_proj rows
  zeroed). Each core computes its heads' QKV columns, attention, and a partial
  row-sliced output projection; the host sums the 8 partials (the Megatron
  row-parallel all-reduce) and transposes back.

All matmuls run as float32r (TF32-class, ~13-bit mantissa, full PE rate at
free-dim >= 256); accumulation is exact fp32 in PSUM.
"""

import numpy as np

import concourse.bacc as bacc
import concourse.mybir as mybir
import concourse.tile as tile
from concourse.bass_utils import run_bass_kernel_spmd

F32 = mybir.dt.float32
F32R = mybir.dt.float32r

T = 1024          # sequence length
C = 768           # channels
NH = 12           # heads
HS = 64           # head size
NCORES = 8
TT = 512          # t-tile (matmul moving free dim)
NT = T // TT      # 2
NCC = C // 128    # 6 contraction chunks
NKC = T // 128    # 8 key chunks
SCALE = 1.0 / 8.0  # 1/sqrt(HS)

# core -> (head0, head1); cores 4-7 duplicate their head (2nd W_proj slice zeroed)
HEAD_MAP = [(0, 1), (2, 3), (4, 5), (6, 7), (8, 8), (9, 9), (10, 10), (11, 11)]

_CACHE: dict = {}


def _build_program():
    nc = bacc.Bacc("TRN2", target_bir_lowering=False, debug=False,
                   num_devices=NCORES)
    xT = nc.dram_tensor("xT", [C, T], F32, kind="ExternalInput").ap()
    wsel = nc.dram_tensor("wsel", [C, 384], F32, kind="ExternalInput").ap()
    wp = nc.dram_tensor("wp", [128, C], F32, kind="ExternalInput").ap()
    bqk = nc.dram_tensor("bqk", [128, 3], F32, kind="ExternalInput").ap()
    bpr = nc.dram_tensor("bpr", [128, NCC], F32, kind="ExternalInput").ap()
    eye2 = nc.dram_tensor("eye2", [128, HS], F32, kind="ExternalInput").ap()
    ones = nc.dram_tensor("ones", [128, 130], F32, kind="ExternalInput").ap()
    outT = nc.dram_tensor("outT", [C, T], F32, kind="ExternalOutput").ap()

    with tile.TileContext(nc) as tc:
        with (
            tc.tile_pool(name="const", bufs=1) as cp,
            tc.tile_pool(name="e", bufs=8) as ep,
            tc.tile_pool(name="rb", bufs=4) as rbp,
            tc.tile_pool(name="pmm", bufs=2, space="PSUM") as pmm,
            tc.tile_pool(name="pst", bufs=3, space="PSUM") as pst,
            tc.tile_pool(name="pov", bufs=2, space="PSUM") as pov,
            tc.tile_pool(name="ptr", bufs=1, space="PSUM") as ptr,
        ):
            # ---- big merged loads on SP; small constants on Pool's queue ----
            wt = cp.tile([128, NCC * 384], F32R, tag="wt")
            xt0 = cp.tile([128, NCC * TT], F32R, tag="xt0")
            xt1 = cp.tile([128, NCC * TT], F32R, tag="xt1")
            wsel3 = wsel.rearrange("(c p) j -> p c j", p=128).bitcast(F32R)
            x03 = xT[:, 0:TT].rearrange("(c p) t -> p c t", p=128).bitcast(F32R)
            x13 = xT[:, TT:T].rearrange("(c p) t -> p c t", p=128).bitcast(F32R)
            # interleave thirds so chunk cc lands early
            for c0, c1 in ((0, 2), (2, 4), (4, 6)):
                nc.sync.dma_start(
                    out=wt[:].rearrange("p (c j) -> p c j", c=NCC)[:, c0:c1],
                    in_=wsel3[:, c0:c1])
                nc.sync.dma_start(
                    out=xt0[:].rearrange("p (c t) -> p c t", c=NCC)[:, c0:c1],
                    in_=x03[:, c0:c1])
            for pc in range(3):
                c0, c1 = pc * 2, pc * 2 + 2
                nc.sync.dma_start(
                    out=xt1[:].rearrange("p (c t) -> p c t", c=NCC)[:, c0:c1],
                    in_=x13[:, c0:c1])
            wpt = cp.tile([64, 2 * C], F32R, tag="wpt")
            nc.sync.dma_start(
                out=wpt[:].rearrange("p (h e) -> p h e", h=2),
                in_=wp.rearrange("(h p) e -> p h e", p=64).bitcast(F32R))
            xts = [[xt0[:, cc * TT:(cc + 1) * TT], xt1[:, cc * TT:(cc + 1) * TT]]
                   for cc in range(NCC)]
            ws = [wt[:, cc * 384:(cc + 1) * 384] for cc in range(NCC)]
            wps = [wpt[:, hi * C:(hi + 1) * C] for hi in range(2)]

            bqk_sb = cp.tile([128, 3], F32, tag="bqk")
            nc.gpsimd.dma_start(out=bqk_sb[:], in_=bqk)
            eye_sb = cp.tile([128, HS], F32R, tag="eye")
            nc.gpsimd.dma_start(out=eye_sb[:], in_=eye2.bitcast(F32R))
            ones_sb = cp.tile([128, 2], F32R, tag="ones_sb")
            nc.gpsimd.dma_start(out=ones_sb[:], in_=ones[:, 0:2].bitcast(F32R))
            vaug = [cp.tile([128, 130], F32R, tag=f"va{kc}", name=f"va{kc}")
                    for kc in range(NKC)]
            for kc in range(NKC):
                # ones columns at 64 and 129 (cols 0:64 / 65:129 overwritten later)
                nc.vector.tensor_copy(
                    vaug[kc][:, 64:130:65], ones_sb[:])
            bpr_sb = cp.tile([128, NCC], F32, tag="bpr")
            nc.gpsimd.dma_start(out=bpr_sb[:], in_=bpr)

            # static causal masks for the DVE half of the mask work
            masks = []
            for kcr in range(4):
                m = cp.tile([128, TT], F32, tag=f"mask{kcr}", name=f"mask{kcr}")
                nc.vector.memset(m[:], 1.0)
                nc.gpsimd.affine_select(
                    m[:], m[:], pattern=[[1, TT]],
                    compare_op=mybir.AluOpType.is_ge, fill=0.0,
                    base=-128 * kcr, channel_multiplier=-1)
                masks.append(m)

            qkvT = [[None] * NT for _ in range(3)]
            yT = [[None] * NT for _ in range(2)]
            ost = [cp.tile([128, 2 * TT], F32, tag=f"ost{tt}{h}", name=f"ost{tt}{h}")
                   for tt in range(NT) for h in range(3)]

            def emit_qkv(tt):
                for mt in (2, 0, 1):
                    qkvT[mt][tt] = cp.tile([128, TT], F32R, tag=f"qkv{mt}_{tt}",
                                           name=f"qkv{mt}_{tt}")
                    ps = pmm.tile([128, TT], F32, tag="mm")
                    for cc in range(NCC):
                        nc.tensor.matmul(
                            ps[:], ws[cc][:, mt * 128:(mt + 1) * 128],
                            xts[cc][tt], start=(cc == 0), stop=(cc == NCC - 1))
                    nc.vector.tensor_scalar_add(
                        qkvT[mt][tt][:], ps[:], bqk_sb[:, mt:mt + 1])

            def emit_vaug(tt):
                for kc in range(tt * 4, tt * 4 + 4):
                    col = (kc % 4) * 128
                    for hi in range(2):
                        pt = ptr.tile([128, HS], F32R, tag="pt")
                        nc.tensor.transpose(
                            pt[:], qkvT[2][tt][hi * 64:(hi + 1) * 64, col:col + 128],
                            eye_sb[hi * 64:(hi + 1) * 64, :])
                        nc.vector.tensor_copy(vaug[kc][:, hi * 65:hi * 65 + 64], pt[:])

            def emit_attn(qt):
                for hi in range(2):
                    nlive = qt * 4 + 4
                    po = pov.tile([65, TT], F32, tag="po")
                    for kc in range(nlive):
                        ktile = qkvT[1][kc // 4]
                        kcol = (kc % 4) * 128
                        ps = pst.tile([128, TT], F32, tag="st")
                        nc.tensor.matmul(
                            ps[:], ktile[hi * 64:(hi + 1) * 64, kcol:kcol + 128],
                            qkvT[0][qt][hi * 64:(hi + 1) * 64, :],
                            start=True, stop=True)
                        e = ep.tile([128, TT], F32R, tag="e")
                        nc.scalar.activation(
                            e[:], ps[:], mybir.ActivationFunctionType.Exp,
                            scale=SCALE)
                        kcr = kc - qt * 4
                        if kcr >= 0:  # diagonal chunk: zero where tk > tq
                            if kcr % 2 == 0:
                                nc.gpsimd.affine_select(
                                    e[:], e[:], pattern=[[1, TT]],
                                    compare_op=mybir.AluOpType.is_ge, fill=0.0,
                                    base=-128 * kcr, channel_multiplier=-1)
                            else:
                                nc.vector.tensor_mul(e[:], e[:], masks[kcr][:])
                        nc.tensor.matmul(
                            po[:], vaug[kc][:, hi * 65:(hi + 1) * 65], e[:],
                            start=(kc == 0), stop=(kc == nlive - 1))
                    rb = rbp.tile([128, TT], F32, tag="rb")
                    nc.vector.reciprocal(rb[0:1, :], po[64:65, :])
                    rbc = rbp.tile([64, TT], F32, tag="rbc")
                    nc.gpsimd.partition_broadcast(rbc[:], rb[0:1, :])
                    yT[hi][qt] = cp.tile([64, TT], F32R, tag=f"y{hi}_{qt}",
                                         name=f"y{hi}_{qt}")
                    nc.vector.tensor_mul(yT[hi][qt][:], po[0:64, :], rbc[:])

            def emit_proj(tt):
                for half in range(3):
                    stile = ost[tt * 3 + half]
                    for ei in range(2):
                        et = half * 2 + ei
                        pm = pmm.tile([128, TT], F32, tag="mm")
                        for hi in range(2):
                            nc.tensor.matmul(
                                pm[:], wps[hi][:, et * 128:(et + 1) * 128],
                                yT[hi][tt][:], start=(hi == 0), stop=(hi == 1))
                        dst = stile[:, ei * TT:(ei + 1) * TT]
                        if et % 2 == 0:
                            nc.scalar.activation(
                                dst, pm[:], mybir.ActivationFunctionType.Identity,
                                bias=bpr_sb[:, et:et + 1])
                        else:
                            nc.vector.tensor_scalar_add(dst, pm[:], bpr_sb[:, et:et + 1])
                    nc.sync.dma_start(
                        out=outT[half * 256:(half + 1) * 256, tt * TT:(tt + 1) * TT]
                        .rearrange("(g p) t -> p g t", p=128),
                        in_=stile[:].rearrange("p (g t) -> p g t", g=2))

            emit_qkv(0)
            emit_vaug(0)
            emit_attn(0)
            emit_qkv(1)
            emit_vaug(1)
            emit_proj(0)
            emit_attn(1)
            emit_proj(1)
    nc.compile()
    return nc


def _in_maps(x, W_attn, b_attn, W_proj, b_proj):
    xTn = np.ascontiguousarray(x.reshape(T, C).T)  # [C, T]
    eye2 = np.ascontiguousarray(np.tile(np.eye(HS, dtype=np.float32), (2, 1)))
    maps = []
    for core in range(NCORES):
        h0, h1 = HEAD_MAP[core]
        cols = []
        for part in range(3):  # q, k, v column groups of W_attn
            for h in (h0, h1):
                cols.extend(range(part * C + h * HS, part * C + (h + 1) * HS))
        wsel = np.ascontiguousarray(W_attn[:, cols])                    # [C, 384]
        bqk = np.stack(
            [np.concatenate([b_attn[p * C + h0 * HS:p * C + (h0 + 1) * HS],
                             b_attn[p * C + h1 * HS:p * C + (h1 + 1) * HS]])
             for p in range(3)], axis=1).astype(np.float32)             # [128, 3]
        wpc = np.concatenate(
            [W_proj[h0 * HS:(h0 + 1) * HS, :],
             np.zeros_like(W_proj[:HS]) if h1 == h0
             else W_proj[h1 * HS:(h1 + 1) * HS, :]], axis=0)            # [128, C]
        bpr = (b_proj.reshape(NCC, 128).T if core == 0
               else np.zeros((128, NCC), np.float32)).astype(np.float32)
        maps.append({
            "xT": xTn, "wsel": np.ascontiguousarray(wsel.astype(np.float32)),
            "wp": np.ascontiguousarray(wpc.astype(np.float32)),
            "bqk": np.ascontiguousarray(bqk), "bpr": np.ascontiguousarray(bpr),
            "eye2": eye2, "ones": np.ones((128, 130), np.float32),
        })
    return maps


def kernel(x, W_attn, b_attn, W_proj, b_proj, _trace=False, _trace_kwargs=None):
    x = np.asarray(x, np.float32)
    W_attn = np.asarray(W_attn, np.float32)
    b_attn = np.asarray(b_attn, np.float32)
    W_proj = np.asarray(W_proj, np.float32)
    b_proj = np.asarray(b_proj, np.float32)

    if "nc" not in _CACHE:
        _CACHE["nc"] = _build_program()
    nc = _CACHE["nc"]

    maps = _in_maps(x, W_attn, b_attn, W_proj, b_proj)
    kw = {}
    if _trace:
        kw = dict(trace=True, **(_trace_kwargs or {}))
    br = run_bass_kernel_spmd(nc, maps, list(range(NCORES)), **kw)
    acc = np.zeros((C, T), np.float64)
    for core in range(NCORES):
        acc += br.results[core]["outT"].astype(np.float64)
    out = np.ascontiguousarray(acc.T.astype(np.float32)).reshape(1, T, C)
    _CACHE["last_results"] = br
    return out



# revision 2
# speedup vs baseline: 1.0256x; 1.0256x over previous
"""TRN2 Bass kernel for nn_CausalSelfAttention_4054449128214.

The reference returns out_s + stop_gradient(out_full - out_s), whose forward
value is exactly out_full — plain dense causal self-attention. So the kernel
computes: qkv = x@W_attn+b_attn, per-head causal softmax attention, y@W_proj+b_proj.

Sharding (8 cores, no collectives):
  Megatron head-parallel. Cores 0-3 own head pairs (0,1)..(6,7); cores 4-7 own
  heads 8..11 (run twice for SPMD shape-uniformity, second copy's W_proj rows
  zeroed). Each core computes its heads' Q/K columns, V^T directly via matmul
  (lhsT = x chunk, rhs = W_v), attention, and a partial row-sliced output
  projection; the host sums the 8 partials (the Megatron row-parallel
  all-reduce) and transposes back.

All matmuls run in bf16 (full PE rate at any free-dim size); accumulation is
exact fp32 in PSUM. Bias algebra: b_k shifts every score of a query by the
same amount -> softmax-invariant -> dropped. b_v and b_proj contribute a
constant per-output-column vector (softmax rows sum to 1) -> added on host.
Only b_q is applied in-kernel.
"""

import numpy as np
import ml_dtypes

import concourse.bacc as bacc
import concourse.mybir as mybir
import concourse.tile as tile
from concourse.bass_utils import run_bass_kernel_spmd

F32 = mybir.dt.float32
BF16 = mybir.dt.bfloat16

T = 1024          # sequence length
C = 768           # channels
NH = 12           # heads
HS = 64           # head size
NCORES = 8
TT = 512          # t-tile (matmul moving free dim)
NT = T // TT      # 2
NCC = C // 128    # 6 contraction chunks
NKC = T // 128    # 8 key chunks
SCALE = 1.0 / 8.0  # 1/sqrt(HS)

# core -> (head0, head1); cores 4-7 duplicate their head (2nd W_proj slice zeroed)
HEAD_MAP = [(0, 1), (2, 3), (4, 5), (6, 7), (8, 8), (9, 9), (10, 10), (11, 11)]

_CACHE: dict = {}


def _build_program():
    nc = bacc.Bacc("TRN2", target_bir_lowering=False, debug=False,
                   num_devices=NCORES)
    xT = nc.dram_tensor("xT", [C, T], BF16, kind="ExternalInput").ap()
    wsel = nc.dram_tensor("wsel", [C, 384], BF16, kind="ExternalInput").ap()
    wp = nc.dram_tensor("wp", [128, C], BF16, kind="ExternalInput").ap()
    bq = nc.dram_tensor("bq", [128, 1], F32, kind="ExternalInput").ap()
    outT = nc.dram_tensor("outT", [C, T], BF16, kind="ExternalOutput").ap()

    with tile.TileContext(nc) as tc:
        with (
            tc.tile_pool(name="const", bufs=1) as cp,
            tc.tile_pool(name="e", bufs=8) as ep,
            tc.tile_pool(name="rb", bufs=4) as rbp,
            tc.tile_pool(name="pmm", bufs=2, space="PSUM") as pmm,
            tc.tile_pool(name="pst", bufs=3, space="PSUM") as pst,
            tc.tile_pool(name="pov", bufs=2, space="PSUM") as pov,
            tc.tile_pool(name="pvt", bufs=1, space="PSUM") as pvt,
        ):
            # ---- interleaved per-chunk loads on SP; small constants on Pool ----
            wt = cp.tile([128, NCC * 384], BF16, tag="wt")
            xt0 = cp.tile([128, NCC * TT], BF16, tag="xt0")
            xt1 = cp.tile([128, NCC * TT], BF16, tag="xt1")
            wsel3 = wsel.rearrange("(c p) j -> p c j", p=128)
            x03 = xT[:, 0:TT].rearrange("(c p) t -> p c t", p=128)
            x13 = xT[:, TT:T].rearrange("(c p) t -> p c t", p=128)
            wt3 = wt[:].rearrange("p (c j) -> p c j", c=NCC)
            xt03 = xt0[:].rearrange("p (c t) -> p c t", c=NCC)
            xt13 = xt1[:].rearrange("p (c t) -> p c t", c=NCC)
            # chunk cc of weights, then chunk cc of x, so compute starts early
            for cc in range(NCC):
                nc.sync.dma_start(out=wt3[:, cc:cc + 1], in_=wsel3[:, cc:cc + 1])
                nc.sync.dma_start(out=xt03[:, cc:cc + 1], in_=x03[:, cc:cc + 1])
            for cc in range(NCC):
                nc.sync.dma_start(out=xt13[:, cc:cc + 1], in_=x13[:, cc:cc + 1])
            wpt = cp.tile([64, 2 * C], BF16, tag="wpt")
            nc.sync.dma_start(
                out=wpt[:].rearrange("p (h e) -> p h e", h=2),
                in_=wp.rearrange("(h p) e -> p h e", p=64))
            xts = [[xt0[:, cc * TT:(cc + 1) * TT], xt1[:, cc * TT:(cc + 1) * TT]]
                   for cc in range(NCC)]
            ws = [wt[:, cc * 384:(cc + 1) * 384] for cc in range(NCC)]
            wps = [wpt[:, hi * C:(hi + 1) * C] for hi in range(2)]

            bq_sb = cp.tile([128, 1], F32, tag="bq")
            nc.gpsimd.dma_start(out=bq_sb[:], in_=bq)

            # V^T tiles: [128 keys, 65*2] with a ones column at 64 and 129
            vaug = [cp.tile([128, 130], BF16, tag=f"va{kc}", name=f"va{kc}")
                    for kc in range(NKC)]
            for kc in range(NKC):
                nc.vector.memset(vaug[kc][:, 64:65], 1.0)
                nc.vector.memset(vaug[kc][:, 129:130], 1.0)

            # static causal masks for the DVE half of the mask work
            masks = []
            for kcr in range(4):
                m = cp.tile([128, TT], BF16, tag=f"mask{kcr}", name=f"mask{kcr}")
                nc.vector.memset(m[:], 1.0)
                nc.gpsimd.affine_select(
                    m[:], m[:], pattern=[[1, TT]],
                    compare_op=mybir.AluOpType.is_ge, fill=0.0,
                    base=-128 * kcr, channel_multiplier=-1)
                masks.append(m)

            qT = [None] * NT   # [128=(2h x 64hs), TT] bf16
            kT = [None] * NT
            yT = [[None] * NT for _ in range(2)]

            def emit_qkv(tt):
                # K first (unbiased copy), then Q (bias add)
                kT[tt] = cp.tile([128, TT], BF16, tag=f"k{tt}", name=f"k{tt}")
                ps = pmm.tile([128, TT], F32, tag="mm")
                for cc in range(NCC):
                    nc.tensor.matmul(
                        ps[:], ws[cc][:, 128:256], xts[cc][tt],
                        start=(cc == 0), stop=(cc == NCC - 1))
                nc.scalar.activation(
                    kT[tt][:], ps[:], mybir.ActivationFunctionType.Copy)
                qT[tt] = cp.tile([128, TT], BF16, tag=f"q{tt}", name=f"q{tt}")
                ps = pmm.tile([128, TT], F32, tag="mm")
                for cc in range(NCC):
                    nc.tensor.matmul(
                        ps[:], ws[cc][:, 0:128], xts[cc][tt],
                        start=(cc == 0), stop=(cc == NCC - 1))
                nc.vector.tensor_scalar_add(qT[tt][:], ps[:], bq_sb[:, 0:1])

            def emit_vaug(tt):
                # V^T directly: out[keys, hs] = sum_c x[c, key] * Wv[c, hs]
                for kc in range(tt * 4, tt * 4 + 4):
                    kcol = (kc % 4) * 128
                    pt = pvt.tile([128, 128], F32, tag="pt")
                    for cc in range(NCC):
                        nc.tensor.matmul(
                            pt[:], xts[cc][tt][:, kcol:kcol + 128],
                            ws[cc][:, 256:384],
                            start=(cc == 0), stop=(cc == NCC - 1))
                    nc.vector.tensor_copy(vaug[kc][:, 0:64], pt[:, 0:64])
                    nc.vector.tensor_copy(vaug[kc][:, 65:129], pt[:, 64:128])

            def emit_attn(qt):
                for hi in range(2):
                    nlive = qt * 4 + 4
                    po = pov.tile([65, TT], F32, tag="po")
                    for kc in range(nlive):
                        ktile = kT[kc // 4]
                        kcol = (kc % 4) * 128
                        ps = pst.tile([128, TT], F32, tag="st")
                        nc.tensor.matmul(
                            ps[:], ktile[hi * 64:(hi + 1) * 64, kcol:kcol + 128],
                            qT[qt][hi * 64:(hi + 1) * 64, :],
                            start=True, stop=True)
                        e = ep.tile([128, TT], BF16, tag="e")
                        nc.scalar.activation(
                            e[:], ps[:], mybir.ActivationFunctionType.Exp,
                            scale=SCALE)
                        kcr = kc - qt * 4
                        if kcr >= 0:  # diagonal chunk: zero where tk > tq
                            if kcr % 2 == 0:
                                nc.gpsimd.affine_select(
                                    e[:], e[:], pattern=[[1, TT]],
                                    compare_op=mybir.AluOpType.is_ge, fill=0.0,
                                    base=-128 * kcr, channel_multiplier=-1)
                            else:
                                nc.vector.tensor_mul(e[:], e[:], masks[kcr][:])
                        nc.tensor.matmul(
                            po[:], vaug[kc][:, hi * 65:(hi + 1) * 65], e[:],
                            start=(kc == 0), stop=(kc == nlive - 1))
                    rb = rbp.tile([1, TT], F32, tag="rb")
                    nc.vector.reciprocal(rb[0:1, :], po[64:65, :])
                    rbc = rbp.tile([64, TT], F32, tag="rbc")
                    nc.gpsimd.partition_broadcast(rbc[:], rb[0:1, :])
                    yT[hi][qt] = cp.tile([64, TT], BF16, tag=f"y{hi}_{qt}",
                                         name=f"y{hi}_{qt}")
                    nc.vector.tensor_mul(yT[hi][qt][:], po[0:64, :], rbc[:])

            def emit_proj(tt):
                for et in range(6):
                    pm = pmm.tile([128, TT], F32, tag="mm")
                    for hi in range(2):
                        nc.tensor.matmul(
                            pm[:], wps[hi][:, et * 128:(et + 1) * 128],
                            yT[hi][tt][:], start=(hi == 0), stop=(hi == 1))
                    ost = cp.tile([128, TT], BF16, tag=f"ost{tt}_{et}",
                                  name=f"ost{tt}_{et}")
                    if et % 2 == 0:
                        nc.scalar.activation(
                            ost[:], pm[:], mybir.ActivationFunctionType.Copy)
                    else:
                        nc.vector.tensor_copy(ost[:], pm[:])
                    nc.sync.dma_start(
                        out=outT[et * 128:(et + 1) * 128, tt * TT:(tt + 1) * TT],
                        in_=ost[:])

            emit_qkv(0)
            emit_vaug(0)
            emit_attn(0)
            emit_qkv(1)
            emit_vaug(1)
            emit_proj(0)
            emit_attn(1)
            emit_proj(1)
    nc.compile()
    return nc


def _in_maps(x, W_attn, b_attn, W_proj, b_proj):
    bf = ml_dtypes.bfloat16
    xTn = np.ascontiguousarray(x.reshape(T, C).T.astype(bf))  # [C, T]
    maps = []
    for core in range(NCORES):
        h0, h1 = HEAD_MAP[core]
        cols = []
        for part in range(3):  # q, k, v column groups of W_attn
            for h in (h0, h1):
                cols.extend(range(part * C + h * HS, part * C + (h + 1) * HS))
        wsel = np.ascontiguousarray(W_attn[:, cols].astype(bf))         # [C, 384]
        bqc = np.concatenate([b_attn[h0 * HS:(h0 + 1) * HS],
                              b_attn[h1 * HS:(h1 + 1) * HS]]
                             ).astype(np.float32).reshape(128, 1)
        wpc = np.concatenate(
            [W_proj[h0 * HS:(h0 + 1) * HS, :],
             np.zeros_like(W_proj[:HS]) if h1 == h0
             else W_proj[h1 * HS:(h1 + 1) * HS, :]], axis=0)            # [128, C]
        maps.append({
            "xT": xTn, "wsel": wsel,
            "wp": np.ascontiguousarray(wpc.astype(bf)),
            "bq": np.ascontiguousarray(bqc),
        })
    return maps


def kernel(x, W_attn, b_attn, W_proj, b_proj, _trace=False, _trace_kwargs=None):
    x = np.asarray(x, np.float32)
    W_attn = np.asarray(W_attn, np.float32)
    b_attn = np.asarray(b_attn, np.float32)
    W_proj = np.asarray(W_proj, np.float32)
    b_proj = np.asarray(b_proj, np.float32)

    if "nc" not in _CACHE:
        _CACHE["nc"] = _build_program()
    nc = _CACHE["nc"]

    maps = _in_maps(x, W_attn, b_attn, W_proj, b_proj)
    kw = {}
    if _trace:
        kw = dict(trace=True, **(_trace_kwargs or {}))
    br = run_bass_kernel_spmd(nc, maps, list(range(NCORES)), **kw)
    acc = np.zeros((C, T), np.float64)
    for core in range(NCORES):
        acc += br.results[core]["outT"].astype(np.float64)
    # host-side bias fold: b_v @ W_proj + b_proj (softmax rows sum to 1)
    bias = (b_attn[2 * C:].astype(np.float64) @ W_proj.astype(np.float64)
            + b_proj.astype(np.float64))
    out = np.ascontiguousarray((acc.T + bias[None, :]).astype(np.float32))
    out = out.reshape(1, T, C)
    _CACHE["last_results"] = br
    return out


# revision 6
# speedup vs baseline: 1.0313x; 1.0056x over previous
"""TRN2 Bass kernel for nn_CausalSelfAttention_4054449128214.

The reference returns out_s + stop_gradient(out_full - out_s), whose forward
value is exactly out_full — plain dense causal self-attention. So the kernel
computes: qkv = x@W_attn+b_attn, per-head causal softmax attention, y@W_proj+b_proj.

Sharding (8 cores, no collectives):
  Megatron head-parallel. Cores 0-3 own head pairs (0,1)..(6,7); cores 4-7 own
  heads 8..11 (run twice for SPMD shape-uniformity, second copy's W_proj rows
  zeroed). Each core computes its heads' Q/K columns, V^T directly via matmul
  (lhsT = x chunk, rhs = W_v), attention, and a partial row-sliced output
  projection; the host sums the 8 partials (the Megatron row-parallel
  all-reduce) and transposes back.

Perf structure:
  - all inputs packed into ONE dram blob, consumption-ordered; per-cc segments
    (wqk|wv|x0) so each DMA unlocks a full contraction-chunk of K/Q/V^T work
    (HWDGE issue is a serial 625ns/DMA resource — DMA count is precious).
  - all matmul operands bf16 (full PE rate, half DMA bytes, 2x DVE rate);
    fp32 PSUM accumulation.
  - K/Q/V^T accumulation groups interleaved per-cc so PE starts on chunk 0.
  - attention (Act-exp-limited) interleaved with the next phase's matmuls:
    attn(qt0) x qkv(tt1), attn(qt1) x proj(tt0).
  - bias algebra: b_k is softmax-invariant (dropped); b_v/b_proj fold into a
    host-side constant column vector (softmax rows sum to 1); only b_q in-kernel.
"""

import numpy as np
import ml_dtypes

import concourse.bacc as bacc
import concourse.mybir as mybir
import concourse.tile as tile
from concourse.bass_utils import run_bass_kernel_spmd

F32 = mybir.dt.float32
BF16 = mybir.dt.bfloat16

T = 1024          # sequence length
C = 768           # channels
NH = 12           # heads
HS = 64           # head size
NCORES = 8
TT = 512          # t-tile (matmul moving free dim)
NT = T // TT      # 2
NCC = C // 128    # 6 contraction chunks
NKC = T // 128    # 8 key chunks
SCALE = 1.0 / 8.0  # 1/sqrt(HS)

SEG = 896                 # per-cc blob segment: wqk(256) | wv(128) | x0(512)
X1OFF = NCC * SEG         # 5376
WPOFF = X1OFF + NCC * TT  # 8448
BLOBW = WPOFF + C         # 9216

# core -> (head0, head1); cores 4-7 duplicate their head (2nd W_proj slice zeroed)
HEAD_MAP = [(0, 1), (2, 3), (4, 5), (6, 7), (8, 8), (9, 9), (10, 10), (11, 11)]

_CACHE: dict = {}


def _build_program():
    nc = bacc.Bacc("TRN2", target_bir_lowering=False, debug=False,
                   num_devices=NCORES)
    blob = nc.dram_tensor("blob", [128, BLOBW], BF16, kind="ExternalInput").ap()
    bq = nc.dram_tensor("bq", [128, 1], F32, kind="ExternalInput").ap()
    outT = nc.dram_tensor("outT", [C, T], BF16, kind="ExternalOutput").ap()

    with tile.TileContext(nc) as tc:
        with (
            tc.tile_pool(name="const", bufs=1) as cp,
            tc.tile_pool(name="e", bufs=8) as ep,
            tc.tile_pool(name="rb", bufs=4) as rbp,
            tc.tile_pool(name="pmm", bufs=2, space="PSUM") as pmm,
            tc.tile_pool(name="pst", bufs=2, space="PSUM") as pst,
            tc.tile_pool(name="pov", bufs=2, space="PSUM") as pov,
            tc.tile_pool(name="pvt", bufs=2, space="PSUM") as pvt,
        ):
            bsb = cp.tile([128, WPOFF], BF16, tag="bsb")
            wpt = cp.tile([64, 2 * C], BF16, tag="wpt")
            for cc in range(NCC):
                nc.sync.dma_start(out=bsb[:, cc * SEG:(cc + 1) * SEG],
                                  in_=blob[:, cc * SEG:(cc + 1) * SEG])
            nc.sync.dma_start(out=bsb[:, X1OFF:X1OFF + 3 * TT],
                              in_=blob[:, X1OFF:X1OFF + 3 * TT])
            nc.sync.dma_start(out=bsb[:, X1OFF + 3 * TT:WPOFF],
                              in_=blob[:, X1OFF + 3 * TT:WPOFF])
            nc.sync.dma_start(
                out=wpt[:].rearrange("p (h e) -> p h e", h=2),
                in_=blob[:, WPOFF:BLOBW].rearrange("(h p) e -> p h e", p=64))
            bq_sb = cp.tile([128, 1], F32, tag="bq")
            nc.gpsimd.dma_start(out=bq_sb[:], in_=bq)

            wqk = [bsb[:, cc * SEG:cc * SEG + 256] for cc in range(NCC)]
            wv = [bsb[:, cc * SEG + 256:cc * SEG + 384] for cc in range(NCC)]
            xts = [[bsb[:, cc * SEG + 384:(cc + 1) * SEG],
                    bsb[:, X1OFF + cc * TT:X1OFF + (cc + 1) * TT]]
                   for cc in range(NCC)]
            wps = [wpt[:, hi * C:(hi + 1) * C] for hi in range(2)]

            # V^T tiles: [128 keys, 65*2] with a ones column at 64 and 129
            vaug = [cp.tile([128, 130], BF16, tag=f"va{kc}", name=f"va{kc}")
                    for kc in range(NKC)]
            for kc in range(NKC):
                nc.vector.memset(vaug[kc][:, 64:130:65], 1.0)

            # static causal masks for the DVE half of the mask work
            masks = []
            for kcr in range(4):
                m = cp.tile([128, TT], BF16, tag=f"mask{kcr}", name=f"mask{kcr}")
                nc.vector.memset(m[:], 1.0)
                nc.gpsimd.affine_select(
                    m[:], m[:], pattern=[[1, TT]],
                    compare_op=mybir.AluOpType.is_ge, fill=0.0,
                    base=-128 * kcr, channel_multiplier=-1)
                masks.append(m)

            qT = [None] * NT   # [128=(2h x 64hs), TT] bf16
            kT = [None] * NT
            yT = [[None] * NT for _ in range(2)]
            st8 = {}           # per-tt K/Q/V01 psum state

            def vcopy(kc, pt):
                nc.vector.tensor_copy(vaug[kc][:, 0:64], pt[:, 0:64])
                nc.vector.tensor_copy(vaug[kc][:, 65:129], pt[:, 64:128])

            def u_cc(tt, cc):
                # one contraction chunk of the K/Q/V0/V1 accumulations
                if cc == 0:
                    st8[tt] = (pmm.tile([128, TT], F32, tag="mm", name=f"psK{tt}"),
                               pmm.tile([128, TT], F32, tag="mm", name=f"psQ{tt}"),
                               pvt.tile([128, 128], F32, tag="pt", name=f"ptA{tt}"),
                               pvt.tile([128, 128], F32, tag="pt", name=f"ptB{tt}"))
                psK, psQ, ptA, ptB = st8[tt]
                st, sp = cc == 0, cc == NCC - 1
                nc.tensor.matmul(psK[:], wqk[cc][:, 128:256], xts[cc][tt],
                                 start=st, stop=sp)
                nc.tensor.matmul(psQ[:], wqk[cc][:, 0:128], xts[cc][tt],
                                 start=st, stop=sp)
                nc.tensor.matmul(ptA[:], xts[cc][tt][:, 0:128], wv[cc],
                                 start=st, stop=sp)
                nc.tensor.matmul(ptB[:], xts[cc][tt][:, 128:256], wv[cc],
                                 start=st, stop=sp)

            def u_fin(tt):
                psK, psQ, ptA, ptB = st8[tt]
                kT[tt] = cp.tile([128, TT], BF16, tag=f"k{tt}", name=f"k{tt}")
                nc.scalar.activation(
                    kT[tt][:], psK[:], mybir.ActivationFunctionType.Copy)
                qT[tt] = cp.tile([128, TT], BF16, tag=f"q{tt}", name=f"q{tt}")
                nc.vector.tensor_scalar_add(qT[tt][:], psQ[:], bq_sb[:, 0:1])
                vcopy(tt * 4 + 0, ptA)
                vcopy(tt * 4 + 1, ptB)

            def u_v23(tt, j):  # j in (2, 3)
                pt = pvt.tile([128, 128], F32, tag="pt")
                for cc in range(NCC):
                    nc.tensor.matmul(pt[:], xts[cc][tt][:, j * 128:(j + 1) * 128],
                                     wv[cc], start=(cc == 0), stop=(cc == NCC - 1))
                vcopy(tt * 4 + j, pt)

            def emit_attn(qt, fillers):
                fit = iter(fillers)
                for hi in range(2):
                    nlive = qt * 4 + 4
                    po = pov.tile([65, TT], F32, tag="po")
                    for kc in range(nlive):
                        ktile = kT[kc // 4]
                        kcol = (kc % 4) * 128
                        ps = pst.tile([128, TT], F32, tag="st")
                        nc.tensor.matmul(
                            ps[:], ktile[hi * 64:(hi + 1) * 64, kcol:kcol + 128],
                            qT[qt][hi * 64:(hi + 1) * 64, :],
                            start=True, stop=True)
                        e = ep.tile([128, TT], BF16, tag="e")
                        nc.scalar.activation(
                            e[:], ps[:], mybir.ActivationFunctionType.Exp,
                            scale=SCALE)
                        kcr = kc - qt * 4
                        if kcr >= 0:  # diagonal chunk: zero where tk > tq
                            if kcr % 2 == 0:
                                nc.gpsimd.affine_select(
                                    e[:], e[:], pattern=[[1, TT]],
                                    compare_op=mybir.AluOpType.is_ge, fill=0.0,
                                    base=-128 * kcr, channel_multiplier=-1)
                            else:
                                nc.vector.tensor_mul(e[:], e[:], masks[kcr][:])
                        nc.tensor.matmul(
                            po[:], vaug[kc][:, hi * 65:(hi + 1) * 65], e[:],
                            start=(kc == 0), stop=(kc == nlive - 1))
                        f = next(fit, None)
                        if f is not None:
                            f()
                    rb = rbp.tile([1, TT], F32, tag="rb")
                    nc.vector.reciprocal(rb[0:1, :], po[64:65, :])
                    rbc = rbp.tile([64, TT], F32, tag="rbc")
                    nc.gpsimd.partition_broadcast(rbc[:], rb[0:1, :])
                    yT[hi][qt] = cp.tile([64, TT], BF16, tag=f"y{hi}_{qt}",
                                         name=f"y{hi}_{qt}")
                    nc.vector.tensor_mul(yT[hi][qt][:], po[0:64, :], rbc[:])
                for f in fit:
                    f()

            osts = {}

            def u_et(tt, et, ost, col):
                # one output-projection column group into ost[:, col*TT:...]
                pm = pmm.tile([128, TT], F32, tag="mm")
                for hi in range(2):
                    nc.tensor.matmul(
                        pm[:], wps[hi][:, et * 128:(et + 1) * 128],
                        yT[hi][tt][:], start=(hi == 0), stop=(hi == 1))
                dst = ost[:, col * TT:(col + 1) * TT]
                if et % 3 == 0:
                    nc.scalar.activation(
                        dst, pm[:], mybir.ActivationFunctionType.Copy)
                else:
                    nc.vector.tensor_copy(dst, pm[:])

            def u_store(tt, et0, ngrp, ost):
                nc.sync.dma_start(
                    out=outT[et0 * 128:(et0 + ngrp) * 128,
                             tt * TT:(tt + 1) * TT]
                    .rearrange("(g p) t -> p g t", p=128),
                    in_=ost[:].rearrange("p (g t) -> p g t", g=ngrp))

            def proj_units(tt, groups):
                units = []
                for et0, ngrp in groups:
                    ost = cp.tile([128, ngrp * TT], BF16,
                                  tag=f"ost{tt}_{et0}", name=f"ost{tt}_{et0}")
                    for i in range(ngrp):
                        units.append(lambda tt=tt, et=et0 + i, ost=ost, col=i:
                                     u_et(tt, et, ost, col))
                    units.append(lambda tt=tt, et0=et0, ngrp=ngrp, ost=ost:
                                 u_store(tt, et0, ngrp, ost))
                return units

            # ---- phase C: qkv+vaug for tt0, DMA-paced per-cc ----
            for cc in range(NCC):
                u_cc(0, cc)
            u_fin(0)
            u_v23(0, 2)
            u_v23(0, 3)
            # ---- phase D: attn(qt0) interleaved with qkv+vaug(tt1) ----
            emit_attn(0, [lambda cc=cc: u_cc(1, cc) for cc in range(NCC)]
                      + [lambda: u_fin(1), lambda: u_v23(1, 2)])
            u_v23(1, 3)
            # ---- phase E: attn(qt1) interleaved with proj(tt0) ----
            pu0 = proj_units(0, [(0, 3), (3, 3)])
            # one proj unit every other attn chunk (8 units over 16 slots)
            emit_attn(1, [f for u in pu0 for f in (u, None)])
            # ---- phase F: proj(tt1) tail, small last store ----
            for u in proj_units(1, [(0, 3), (3, 2), (5, 1)]):
                u()
    nc.compile()
    return nc


def _in_maps(x, W_attn, b_attn, W_proj, b_proj):
    bf = ml_dtypes.bfloat16
    xTn = x.reshape(T, C).T.astype(np.float32)  # [C, T]
    maps = []
    for core in range(NCORES):
        h0, h1 = HEAD_MAP[core]
        cols = []
        for part in range(3):  # q, k, v column groups of W_attn
            for h in (h0, h1):
                cols.extend(range(part * C + h * HS, part * C + (h + 1) * HS))
        wsel = W_attn[:, cols].astype(np.float32)                       # [C, 384]
        bqc = np.concatenate([b_attn[h0 * HS:(h0 + 1) * HS],
                              b_attn[h1 * HS:(h1 + 1) * HS]]
                             ).astype(np.float32).reshape(128, 1)
        wpc = np.concatenate(
            [W_proj[h0 * HS:(h0 + 1) * HS, :],
             np.zeros_like(W_proj[:HS]) if h1 == h0
             else W_proj[h1 * HS:(h1 + 1) * HS, :]], axis=0)            # [128, C]
        blob = np.empty((128, BLOBW), np.float32)
        for cc in range(NCC):
            r = slice(cc * 128, (cc + 1) * 128)
            seg = cc * SEG
            blob[:, seg:seg + 256] = wsel[r, 0:256]
            blob[:, seg + 256:seg + 384] = wsel[r, 256:384]
            blob[:, seg + 384:seg + 896] = xTn[r, 0:TT]
            blob[:, X1OFF + cc * TT:X1OFF + (cc + 1) * TT] = xTn[r, TT:T]
        blob[:, WPOFF:BLOBW] = wpc
        maps.append({
            "blob": np.ascontiguousarray(blob.astype(bf)),
            "bq": np.ascontiguousarray(bqc),
        })
    return maps


def kernel(x, W_attn, b_attn, W_proj, b_proj, _trace=False, _trace_kwargs=None):
    x = np.asarray(x, np.float32)
    W_attn = np.asarray(W_attn, np.float32)
    b_attn = np.asarray(b_attn, np.float32)
    W_proj = np.asarray(W_proj, np.float32)
    b_proj = np.asarray(b_proj, np.float32)

    if "nc" not in _CACHE:
        _CACHE["nc"] = _build_program()
    nc = _CACHE["nc"]

    maps = _in_maps(x, W_attn, b_attn, W_proj, b_proj)
    kw = {}
    if _trace:
        kw = dict(trace=True, **(_trace_kwargs or {}))
    br = run_bass_kernel_spmd(nc, maps, list(range(NCORES)), **kw)
    acc = np.zeros((C, T), np.float64)
    for core in range(NCORES):
        acc += br.results[core]["outT"].astype(np.float64)
    # host-side bias fold: b_v @ W_proj + b_proj (softmax rows sum to 1)
    bias = (b_attn[2 * C:].astype(np.float64) @ W_proj.astype(np.float64)
            + b_proj.astype(np.float64))
    out = np.ascontiguousarray((acc.T + bias[None, :]).astype(np.float32))
    out = out.reshape(1, T, C)
    _CACHE["last_results"] = br
    return out


# revision 13
# speedup vs baseline: 1.1193x; 1.0853x over previous
"""TRN2 Bass kernel for nn_CausalSelfAttention_4054449128214.

The reference returns out_s + stop_gradient(out_full - out_s), whose forward
value is exactly out_full — plain dense causal self-attention. So the kernel
computes: qkv = x@W_attn+b_attn, per-head causal softmax attention, y@W_proj+b_proj.

Sharding (8 cores, no collectives):
  Megatron head-parallel. Cores 0-3 own head pairs (0,1)..(6,7); cores 4-7 own
  heads 8..11 (run twice for SPMD shape-uniformity, second copy's W_proj rows
  zeroed). Each core computes its heads' Q/K columns, V^T directly via matmul
  (lhsT = x chunk, rhs = W_v), attention, and a partial row-sliced output
  projection; the host sums the 8 partials (the Megatron row-parallel
  all-reduce) and transposes back.

Perf structure:
  - all inputs packed into ONE dram blob, consumption-ordered; per-cc segments
    (wqk|wv|x0) so each DMA unlocks a full contraction-chunk of K/Q/V^T work
    (HWDGE issue is a serial 625ns/DMA resource — DMA count is precious).
  - all matmul operands bf16 (full PE rate, half DMA bytes, 2x DVE rate);
    fp32 PSUM accumulation.
  - K/Q/V^T accumulation groups interleaved per-cc so PE starts on chunk 0.
  - attention (Act-exp-limited) interleaved with the next phase's matmuls:
    attn(qt0) x qkv(tt1), attn(qt1) x proj(tt0).
  - bias algebra: b_k is softmax-invariant (dropped); b_v/b_proj fold into a
    host-side constant column vector (softmax rows sum to 1); only b_q in-kernel.
"""

import numpy as np
import ml_dtypes

import concourse.bacc as bacc
import concourse.mybir as mybir
import concourse.tile as tile
from concourse.bass_utils import run_bass_kernel_spmd

F32 = mybir.dt.float32
BF16 = mybir.dt.bfloat16

T = 1024          # sequence length
C = 768           # channels
NH = 12           # heads
HS = 64           # head size
NCORES = 8
TT = 512          # t-tile (matmul moving free dim)
NT = T // TT      # 2
NCC = C // 128    # 6 contraction chunks
NKC = T // 128    # 8 key chunks
SCALE = 1.0 / 8.0  # 1/sqrt(HS)

NDUMMY = 400              # PE p-state warmup matmuls (run during initial DMA wait)
SEG = 896                 # per-cc blob segment: wqk(256) | wv(128) | x0(512)
X1OFF = NCC * SEG         # 5376
WPOFF = X1OFF + NCC * TT  # 8448
BLOBW = WPOFF + C         # 9216

# core -> (head0, head1); cores 4-7 duplicate their head (2nd W_proj slice zeroed)
HEAD_MAP = [(0, 1), (2, 3), (4, 5), (6, 7), (8, 8), (9, 9), (10, 10), (11, 11)]

_CACHE: dict = {}


def _build_program():
    nc = bacc.Bacc("TRN2", target_bir_lowering=False, debug=False,
                   num_devices=NCORES)
    blob = nc.dram_tensor("blob", [128, BLOBW], BF16, kind="ExternalInput").ap()
    bq = nc.dram_tensor("bq", [128, 1], F32, kind="ExternalInput").ap()
    outT = nc.dram_tensor("outT", [C, T], BF16, kind="ExternalOutput").ap()

    with tile.TileContext(nc) as tc:
        with (
            tc.tile_pool(name="const", bufs=1) as cp,
            tc.tile_pool(name="e", bufs=8) as ep,
            tc.tile_pool(name="rb", bufs=4) as rbp,
            tc.tile_pool(name="pmm", bufs=2, space="PSUM") as pmm,
            tc.tile_pool(name="pst", bufs=2, space="PSUM") as pst,
            tc.tile_pool(name="pov", bufs=2, space="PSUM") as pov,
            tc.tile_pool(name="pvt", bufs=2, space="PSUM") as pvt,
        ):
            bsb = cp.tile([128, WPOFF], BF16, tag="bsb")
            wpt = cp.tile([64, 2 * C], BF16, tag="wpt")
            for cc in range(NCC):
                nc.sync.dma_start(out=bsb[:, cc * SEG:(cc + 1) * SEG],
                                  in_=blob[:, cc * SEG:(cc + 1) * SEG])
            nc.sync.dma_start(out=bsb[:, X1OFF:X1OFF + 3 * TT],
                              in_=blob[:, X1OFF:X1OFF + 3 * TT])
            nc.sync.dma_start(out=bsb[:, X1OFF + 3 * TT:WPOFF],
                              in_=blob[:, X1OFF + 3 * TT:WPOFF])
            nc.sync.dma_start(
                out=wpt[:].rearrange("p (h e) -> p h e", h=2),
                in_=blob[:, WPOFF:BLOBW].rearrange("(h p) e -> p h e", p=64))
            bq_sb = cp.tile([128, 1], F32, tag="bq")
            nc.gpsimd.dma_start(out=bq_sb[:], in_=bq)

            wqk = [bsb[:, cc * SEG:cc * SEG + 256] for cc in range(NCC)]
            wv = [bsb[:, cc * SEG + 256:cc * SEG + 384] for cc in range(NCC)]
            xts = [[bsb[:, cc * SEG + 384:(cc + 1) * SEG],
                    bsb[:, X1OFF + cc * TT:X1OFF + (cc + 1) * TT]]
                   for cc in range(NCC)]
            wps = [wpt[:, hi * C:(hi + 1) * C] for hi in range(2)]

            # V^T tiles: [128 keys, 65*2] with a ones column at 64 and 129
            vaug = [cp.tile([128, 130], BF16, tag=f"va{kc}", name=f"va{kc}")
                    for kc in range(NKC)]
            for kc in range(NKC):
                nc.vector.memset(vaug[kc][:, 64:130:65], 1.0)
            ones64 = cp.tile([1, 64], BF16, tag="ones64")
            nc.vector.memset(ones64[:], 1.0)
            # PE warmup: tiny matmuls during the initial DMA wait keep the
            # tensor engine's p-state ramping so real work runs at full clock
            pdum = pvt.tile([1, 1], F32, tag="pt", name="pdum")
            for _ in range(NDUMMY):
                nc.tensor.matmul(pdum[:], ones64[0:1, 0:1], ones64[0:1, 0:1],
                                 start=True, stop=True)

            # static causal masks for the DVE half of the mask work
            masks = []
            for kcr in range(4):
                m = cp.tile([128, TT], BF16, tag=f"mask{kcr}", name=f"mask{kcr}")
                nc.vector.memset(m[:], 1.0)
                nc.gpsimd.affine_select(
                    m[:], m[:], pattern=[[1, TT]],
                    compare_op=mybir.AluOpType.is_ge, fill=0.0,
                    base=-128 * kcr, channel_multiplier=-1)
                masks.append(m)

            qT = [None] * NT   # [128=(2h x 64hs), TT] bf16
            kT = [None] * NT
            yT = [[None] * NT for _ in range(2)]
            st8 = {}           # per-tt K/Q/V01 psum state

            def vcopy(kc, pt):
                nc.vector.tensor_copy(vaug[kc][:, 0:64], pt[:, 0:64])
                nc.vector.tensor_copy(vaug[kc][:, 65:129], pt[:, 64:128])

            def u_cc(tt, cc):
                # one contraction chunk of the K/Q/V0/V1 accumulations
                if cc == 0:
                    st8[tt] = (pmm.tile([128, TT], F32, tag="mm", name=f"psK{tt}"),
                               pmm.tile([128, TT], F32, tag="mm", name=f"psQ{tt}"),
                               pvt.tile([128, 128], F32, tag="pt", name=f"ptA{tt}"),
                               pvt.tile([128, 128], F32, tag="pt", name=f"ptB{tt}"))
                psK, psQ, ptA, ptB = st8[tt]
                st, sp = cc == 0, cc == NCC - 1
                nc.tensor.matmul(psK[:], wqk[cc][:, 128:256], xts[cc][tt],
                                 start=st, stop=sp)
                nc.tensor.matmul(psQ[:], wqk[cc][:, 0:128], xts[cc][tt],
                                 start=st, stop=sp)
                nc.tensor.matmul(ptA[:], xts[cc][tt][:, 0:128], wv[cc],
                                 start=st, stop=sp)
                nc.tensor.matmul(ptB[:], xts[cc][tt][:, 128:256], wv[cc],
                                 start=st, stop=sp)

            def u_fin(tt):
                psK, psQ, ptA, ptB = st8[tt]
                kT[tt] = cp.tile([128, TT], BF16, tag=f"k{tt}", name=f"k{tt}")
                nc.scalar.activation(
                    kT[tt][:], psK[:], mybir.ActivationFunctionType.Copy)
                qT[tt] = cp.tile([128, TT], BF16, tag=f"q{tt}", name=f"q{tt}")
                nc.vector.tensor_scalar_add(qT[tt][:], psQ[:], bq_sb[:, 0:1])
                vcopy(tt * 4 + 0, ptA)
                vcopy(tt * 4 + 1, ptB)

            def u_v23(tt, j):  # j in (2, 3)
                pt = pvt.tile([128, 128], F32, tag="pt")
                for cc in range(NCC):
                    nc.tensor.matmul(pt[:], xts[cc][tt][:, j * 128:(j + 1) * 128],
                                     wv[cc], start=(cc == 0), stop=(cc == NCC - 1))
                vcopy(tt * 4 + j, pt)

            def emit_attn(qt, fillers):
                fit = iter(fillers)
                # masked (diagonal) chunks first so the last chunk's chain is
                # QK->exp->PV with no mask step; order is math-irrelevant (sum)
                kcs = list(range(qt * 4, qt * 4 + 4)) + list(range(0, qt * 4))
                for hi in range(2):
                    po = pov.tile([65, TT], F32, tag="po")
                    for i, kc in enumerate(kcs):
                        ktile = kT[kc // 4]
                        kcol = (kc % 4) * 128
                        ps = pst.tile([128, TT], F32, tag="st")
                        nc.tensor.matmul(
                            ps[:], ktile[hi * 64:(hi + 1) * 64, kcol:kcol + 128],
                            qT[qt][hi * 64:(hi + 1) * 64, :],
                            start=True, stop=True)
                        e = ep.tile([128, TT], BF16, tag="e")
                        nc.scalar.activation(
                            e[:], ps[:], mybir.ActivationFunctionType.Exp,
                            scale=SCALE)
                        kcr = kc - qt * 4
                        if kcr >= 0:  # diagonal chunk: zero where tk > tq
                            if kcr % 2 == 0:
                                nc.gpsimd.affine_select(
                                    e[:], e[:], pattern=[[1, TT]],
                                    compare_op=mybir.AluOpType.is_ge, fill=0.0,
                                    base=-128 * kcr, channel_multiplier=-1)
                            else:
                                nc.vector.tensor_mul(e[:], e[:], masks[kcr][:])
                        nc.tensor.matmul(
                            po[:], vaug[kc][:, hi * 65:(hi + 1) * 65], e[:],
                            start=(i == 0), stop=(i == len(kcs) - 1))
                        f = next(fit, None)
                        if f is not None:
                            f()
                    rb = rbp.tile([1, TT], BF16, tag="rb")
                    with nc.allow_low_precision(
                            reason="bf16 1/s is a uniform 0.4% softmax-scale "
                                   "wobble, within the 2e-2 gate"):
                        nc.vector.reciprocal(rb[0:1, :], po[64:65, :])
                    rbc = rbp.tile([64, TT], BF16, tag="rbc")
                    nc.gpsimd.partition_broadcast(rbc[:], rb[0:1, :])
                    yT[hi][qt] = cp.tile([64, TT], BF16, tag=f"y{hi}_{qt}",
                                         name=f"y{hi}_{qt}")
                    nc.vector.tensor_mul(yT[hi][qt][:], po[0:64, :], rbc[:])
                for f in fit:
                    f()

            osts = {}

            def u_et(tt, et, ost, col):
                # one output-projection column group into ost[:, col*TT:...]
                pm = pmm.tile([128, TT], F32, tag="mm")
                for hi in range(2):
                    nc.tensor.matmul(
                        pm[:], wps[hi][:, et * 128:(et + 1) * 128],
                        yT[hi][tt][:], start=(hi == 0), stop=(hi == 1))
                dst = ost[:, col * TT:(col + 1) * TT]
                # alternate engines so back-to-back tail copies overlap
                on_act = (et % 3 == 0) if tt == 0 else (et % 2 == 1)
                if on_act:
                    nc.scalar.activation(
                        dst, pm[:], mybir.ActivationFunctionType.Copy)
                else:
                    nc.vector.tensor_copy(dst, pm[:])

            def u_store(tt, et0, ngrp, ost):
                nc.sync.dma_start(
                    out=outT[et0 * 128:(et0 + ngrp) * 128,
                             tt * TT:(tt + 1) * TT]
                    .rearrange("(g p) t -> p g t", p=128),
                    in_=ost[:].rearrange("p (g t) -> p g t", g=ngrp))

            def proj_units(tt, groups):
                units = []
                for et0, ngrp in groups:
                    ost = cp.tile([128, ngrp * TT], BF16,
                                  tag=f"ost{tt}_{et0}", name=f"ost{tt}_{et0}")
                    for i in range(ngrp):
                        units.append(lambda tt=tt, et=et0 + i, ost=ost, col=i:
                                     u_et(tt, et, ost, col))
                    units.append(lambda tt=tt, et0=et0, ngrp=ngrp, ost=ost:
                                 u_store(tt, et0, ngrp, ost))
                return units

            # ---- phase C: qkv+vaug for tt0, DMA-paced per-cc ----
            for cc in range(NCC):
                u_cc(0, cc)
            u_fin(0)
            u_v23(0, 2)
            u_v23(0, 3)
            # ---- phase D: attn(qt0) interleaved with qkv+vaug(tt1) ----
            emit_attn(0, [lambda cc=cc: u_cc(1, cc) for cc in range(NCC)]
                      + [lambda: u_fin(1), lambda: u_v23(1, 2)])
            u_v23(1, 3)
            # ---- phase E: attn(qt1) interleaved with proj(tt0) ----
            pu0 = proj_units(0, [(0, 3), (3, 3)])
            # one proj unit every other attn chunk (8 units over 16 slots)
            emit_attn(1, [f for u in pu0 for f in (u, None)])
            # ---- phase F: proj(tt1) tail, small last store ----
            for u in proj_units(1, [(0, 3), (3, 2), (5, 1)]):
                u()
    nc.compile()
    return nc


def _in_maps(x, W_attn, b_attn, W_proj, b_proj):
    bf = ml_dtypes.bfloat16
    xTn = x.reshape(T, C).T.astype(np.float32)  # [C, T]
    maps = []
    for core in range(NCORES):
        h0, h1 = HEAD_MAP[core]
        cols = []
        for part in range(3):  # q, k, v column groups of W_attn
            for h in (h0, h1):
                cols.extend(range(part * C + h * HS, part * C + (h + 1) * HS))
        wsel = W_attn[:, cols].astype(np.float32)                       # [C, 384]
        bqc = np.concatenate([b_attn[h0 * HS:(h0 + 1) * HS],
                              b_attn[h1 * HS:(h1 + 1) * HS]]
                             ).astype(np.float32).reshape(128, 1)
        wpc = np.concatenate(
            [W_proj[h0 * HS:(h0 + 1) * HS, :],
             np.zeros_like(W_proj[:HS]) if h1 == h0
             else W_proj[h1 * HS:(h1 + 1) * HS, :]], axis=0)            # [128, C]
        blob = np.empty((128, BLOBW), np.float32)
        for cc in range(NCC):
            r = slice(cc * 128, (cc + 1) * 128)
            seg = cc * SEG
            blob[:, seg:seg + 256] = wsel[r, 0:256]
            blob[:, seg + 256:seg + 384] = wsel[r, 256:384]
            blob[:, seg + 384:seg + 896] = xTn[r, 0:TT]
            blob[:, X1OFF + cc * TT:X1OFF + (cc + 1) * TT] = xTn[r, TT:T]
        blob[:, WPOFF:BLOBW] = wpc
        maps.append({
            "blob": np.ascontiguousarray(blob.astype(bf)),
            "bq": np.ascontiguousarray(bqc),
        })
    return maps


def kernel(x, W_attn, b_attn, W_proj, b_proj, _trace=False, _trace_kwargs=None):
    x = np.asarray(x, np.float32)
    W_attn = np.asarray(W_attn, np.float32)
    b_attn = np.asarray(b_attn, np.float32)
    W_proj = np.asarray(W_proj, np.float32)
    b_proj = np.asarray(b_proj, np.float32)

    if "nc" not in _CACHE:
        _CACHE["nc"] = _build_program()
    nc = _CACHE["nc"]

    maps = _in_maps(x, W_attn, b_attn, W_proj, b_proj)
    kw = {}
    if _trace:
        kw = dict(trace=True, **(_trace_kwargs or {}))
    br = run_bass_kernel_spmd(nc, maps, list(range(NCORES)), **kw)
    acc = np.zeros((C, T), np.float64)
    for core in range(NCORES):
        acc += br.results[core]["outT"].astype(np.float64)
    # host-side bias fold: b_v @ W_proj + b_proj (softmax rows sum to 1)
    bias = (b_attn[2 * C:].astype(np.float64) @ W_proj.astype(np.float64)
            + b_proj.astype(np.float64))
    out = np.ascontiguousarray((acc.T + bias[None, :]).astype(np.float32))
    out = out.reshape(1, T, C)
    _CACHE["last_results"] = br
    return out
